# revision 12
# baseline (speedup 1.0000x reference)
"""Cross-MultiAttention Trainium2 kernel.

Reference computation (nn_Cross_MultiAttention): two [8,6,128,128] images are
split into 16x16 blocks (B'=512 independent blocks of S=256 tokens, C=6
channels), embedded to EMB=512, cross-attended (two query sets vs a shared
K/V from the concatenated features, 8 heads, depth 64, scale EMB^-0.5), the
two attention outputs are concatenated channel-wise and projected back to 6
channels with a 1x1 conv, then blocks are reassembled.

Strategy: data-parallel over blocks — 64 blocks per NeuronCore x 8 cores.
All device compute is bf16-in/fp32-accumulate on the TensorEngine. Layouts
are feature-major ("transposed") end-to-end so no data transposes are needed
until the final projection:
  e^T = Wemb'^T @ x'      (x' carries a ones-row => embedding bias free;
                           the two images share Wemb so they ride one
                           N=512 moving operand)
  Q^T/K^T feature-major; V token-major with a ones-column per head so the
  attention-value matmul also emits the softmax denominator.
  scores^T = K_h^T' @ Q_h — head pairs issued into disjoint PE row groups
  (tile_position (0,0)/(64,0)) so two K=64 matmuls share the array.
  exp on ScalarE (scale folded in), one op per head-pair.
  O = E^T V' (q-major) -> per-partition reciprocal+scale on VectorE.
  concat -> DMA-engine transposes -> out^T = Wp^T-chunks @ cat^T.
Biases bq/bk (per-partition in feature-major) and bp ride the PSUM->SBUF
copies as tensor_scalar adds; bv (per-free in token-major) is a
tensor_tensor add against a broadcast tile. All are exact.
"""

import numpy as np
import ml_dtypes

import concourse.bass as bass
import concourse.mybir as mybir
import concourse.tile as tile
from concourse import bacc
from concourse.bass_utils import run_bass_kernel_spmd

BLK = 16
EMB = 512
HEADS = 8
DEPTH = 64
S = 256  # tokens per block (16*16)
SCALE = EMB ** (-0.5)
NBLK = 64  # blocks per core
NCORES = 8

BF16 = mybir.dt.bfloat16
F32 = mybir.dt.float32
AF = mybir.ActivationFunctionType

DMA_TRANSPOSE = True  # cat->cat^T on DMA engines instead of the PE array


def _build():
    nc = bacc.Bacc(None)

    # ---- DRAM parameters (per core) ----
    x12_d = nc.declare_dram_parameter("x12", [NBLK, 7, 2 * S], BF16, isOutput=False)
    xc_d = nc.declare_dram_parameter("xc", [NBLK, 13, S], BF16, isOutput=False)
    wq_d = nc.declare_dram_parameter("wq", [128, 4 * EMB], BF16, isOutput=False)
    wk_d = nc.declare_dram_parameter("wk", [128, 4 * EMB], BF16, isOutput=False)
    wv_d = nc.declare_dram_parameter("wv", [128, 4 * EMB], BF16, isOutput=False)
    we1_d = nc.declare_dram_parameter("we1", [7, EMB], BF16, isOutput=False)
    we2_d = nc.declare_dram_parameter("we2", [13, EMB], BF16, isOutput=False)
    wpt_d = nc.declare_dram_parameter("wpt", [128, 48], BF16, isOutput=False)
    bqk_d = nc.declare_dram_parameter("bqk", [128, 8], F32, isOutput=False)
    bvb_d = nc.declare_dram_parameter("bvb", [128, EMB], F32, isOutput=False)
    bpc_d = nc.declare_dram_parameter("bpc", [6, 1], F32, isOutput=False)
    id_d = nc.declare_dram_parameter("ident", [128, 128], BF16, isOutput=False)
    out_d = nc.declare_dram_parameter("out", [NBLK, 6, S], F32, isOutput=True)

    with tile.TileContext(nc) as tc:
        with (
            tc.tile_pool(name="const", bufs=1) as constp,
            tc.tile_pool(name="xin", bufs=3) as xinp,
            tc.tile_pool(name="ebuf", bufs=3) as ebufp,
            tc.tile_pool(name="qkbuf", bufs=3) as qkbufp,
            tc.tile_pool(name="vbuf", bufs=2) as vbufp,
            tc.tile_pool(name="Ebuf", bufs=3) as Ebufp,
            tc.tile_pool(name="catbuf", bufs=3) as catbufp,
            tc.tile_pool(name="ctbuf", bufs=2) as ctbufp,
            tc.tile_pool(name="rbuf", bufs=4) as rbufp,
            tc.tile_pool(name="obuf", bufs=3) as obufp,
            tc.tile_pool(name="psA", bufs=2, space="PSUM") as psAp,
            tc.tile_pool(name="psS", bufs=2, space="PSUM") as psSp,
        ):
            # ---- constants into SBUF ----
            wq_sb = constp.tile([128, 4 * EMB], BF16, tag="wq")
            wk_sb = constp.tile([128, 4 * EMB], BF16, tag="wk")
            wv_sb = constp.tile([128, 4 * EMB], BF16, tag="wv")
            we1_sb = constp.tile([7, EMB], BF16, tag="we1")
            we2_sb = constp.tile([13, EMB], BF16, tag="we2")
            wpt_sb = constp.tile([128, 48], BF16, tag="wpt")
            bqk_sb = constp.tile([128, 8], F32, tag="bqk")
            bvb_sb = constp.tile([128, EMB], F32, tag="bvb")
            bpc_sb = constp.tile([6, 1], F32, tag="bpc")
            id_sb = constp.tile([128, 128], BF16, tag="ident")

            nc.scalar.dma_start(out=wq_sb[:], in_=wq_d[:])
            nc.scalar.dma_start(out=wk_sb[:], in_=wk_d[:])
            nc.scalar.dma_start(out=wv_sb[:], in_=wv_d[:])
            nc.scalar.dma_start(out=we1_sb[:], in_=we1_d[:])
            nc.scalar.dma_start(out=we2_sb[:], in_=we2_d[:])
            nc.scalar.dma_start(out=wpt_sb[:], in_=wpt_d[:])
            nc.scalar.dma_start(out=bqk_sb[:], in_=bqk_d[:])
            nc.scalar.dma_start(out=bvb_sb[:], in_=bvb_d[:])
            nc.scalar.dma_start(out=bpc_sb[:], in_=bpc_d[:])
            nc.scalar.dma_start(out=id_sb[:], in_=id_d[:])

            for bp_ in range(NBLK // 2):  # block pairs (projection batched)
                ct_sb = ctbufp.tile([128, 8 * 2 * S], BF16, tag="ct")
                for bo in range(2):
                    b = 2 * bp_ + bo
                    x12_sb = xinp.tile([7, 2 * S], BF16, tag="x12")
                    xc_sb = xinp.tile([13, S], BF16, tag="xc")
                    nc.scalar.dma_start(out=x12_sb[:], in_=x12_d[b])
                    nc.scalar.dma_start(out=xc_sb[:], in_=xc_d[b])

                    # ---- embeddings (feature-major) ----
                    # e12 chunk k = [e1_k | e2_k] (the two images share Wemb)
                    e12_sb = ebufp.tile([128, 4 * 2 * S], BF16, tag="e12")
                    for half in range(2):
                        ps = psAp.tile([128, 2 * 2 * S], F32, tag="psA")
                        for mm in range(2):
                            m = 2 * half + mm
                            nc.tensor.matmul(
                                ps[:, mm * 2 * S:(mm + 1) * 2 * S],
                                we1_sb[:, m * 128:(m + 1) * 128],
                                x12_sb[:],
                                start=True,
                                stop=True,
                            )
                        if half == 0:
                            nc.scalar.activation(
                                e12_sb[:, half * 4 * S:(half + 1) * 4 * S],
                                ps[:], AF.Copy,
                            )
                        else:
                            nc.vector.tensor_copy(
                                e12_sb[:, half * 4 * S:(half + 1) * 4 * S], ps[:]
                            )
                    ec_sb = ebufp.tile([128, 4 * S], BF16, tag="ec")
                    psc = psAp.tile([128, 2 * 2 * S], F32, tag="psA")
                    for m in range(4):
                        nc.tensor.matmul(
                            psc[:, m * S:(m + 1) * S],
                            we2_sb[:, m * 128:(m + 1) * 128],
                            xc_sb[:],
                            start=True,
                            stop=True,
                        )
                    nc.scalar.activation(ec_sb[:], psc[:], AF.Copy)

                    # ---- Q1|Q2 (feature-major), K (feature-major) ----
                    q12_sb = qkbufp.tile([128, 4 * 2 * S], BF16, tag="q12")
                    for half in range(2):
                        ps = psAp.tile([128, 2 * 2 * S], F32, tag="psA")
                        for mm in range(2):
                            m = 2 * half + mm
                            for k in range(4):
                                nc.tensor.matmul(
                                    ps[:, mm * 2 * S:(mm + 1) * 2 * S],
                                    wq_sb[:, k * EMB + m * 128:
                                          k * EMB + (m + 1) * 128],
                                    e12_sb[:, k * 2 * S:(k + 1) * 2 * S],
                                    start=(k == 0),
                                    stop=(k == 3),
                                )
                        for mm in range(2):
                            m = 2 * half + mm
                            nc.vector.tensor_scalar_add(
                                q12_sb[:, m * 2 * S:(m + 1) * 2 * S],
                                ps[:, mm * 2 * S:(mm + 1) * 2 * S],
                                bqk_sb[:, m:m + 1],
                            )

                    k_sb = qkbufp.tile([128, 4 * S], BF16, tag="k")
                    psk = psAp.tile([128, 2 * 2 * S], F32, tag="psA")
                    for m in range(4):
                        for k in range(4):
                            nc.tensor.matmul(
                                psk[:, m * S:(m + 1) * S],
                                wk_sb[:, k * EMB + m * 128: k * EMB + (m + 1) * 128],
                                ec_sb[:, k * S:(k + 1) * S],
                                start=(k == 0),
                                stop=(k == 3),
                            )
                    for m in range(4):
                        nc.vector.tensor_scalar_add(
                            k_sb[:, m * S:(m + 1) * S],
                            psk[:, m * S:(m + 1) * S],
                            bqk_sb[:, 4 + m:5 + m],
                        )

                    # ---- V token-major, ones column per head ----
                    psV = psAp.tile([128, 2 * 2 * S], F32, tag="psA")
                    for t in range(2):
                        for k in range(4):
                            nc.tensor.matmul(
                                psV[:, t * EMB:(t + 1) * EMB],
                                ec_sb[:, k * S + t * 128: k * S + t * 128 + 128],
                                wv_sb[:, k * EMB:(k + 1) * EMB],
                                start=(k == 0),
                                stop=(k == 3),
                            )
                    vp_sb = vbufp.tile([128, 2 * 520], BF16, tag="vp")
                    nc.vector.memset(
                        vp_sb[:].rearrange(
                            "p (t h c) -> p t h c", t=2, h=8
                        )[:, :, :, 64],
                        1.0,
                    )
                    for t in range(2):
                        nc.vector.tensor_add(
                            vp_sb[:, t * 520:(t + 1) * 520].rearrange(
                                "p (h c) -> p h c", c=65
                            )[:, :, 0:64],
                            psV[:, t * EMB:(t + 1) * EMB].rearrange(
                                "p (h c) -> p h c", c=64
                            ),
                            bvb_sb[:].rearrange("p (h c) -> p h c", c=64),
                        )

                    # ---- attention: head pairs in disjoint PE row groups,
                    # software-pipelined: scores/exp of pair N+1 issue
                    # before the attention-value matmuls of pair N ----
                    cat0 = catbufp.tile([128, 2 * EMB], BF16, tag="cat0")
                    cat1 = catbufp.tile([128, 2 * EMB], BF16, tag="cat1")
                    cats = (cat0, cat1)

                    def emit_scores(p, hp):
                        c = hp  # feature chunk index = h//2
                        psS = psSp.tile([128, 4 * S], F32, tag="psS")
                        for kk in range(2):
                            for ho in range(2):
                                r0 = ho * 64
                                nc.tensor.matmul(
                                    psS[:, ho * 2 * S + kk * S:
                                        ho * 2 * S + (kk + 1) * S],
                                    k_sb[r0:r0 + 64,
                                         c * S + kk * 128: c * S + (kk + 1) * 128],
                                    q12_sb[r0:r0 + 64,
                                           c * 2 * S + p * S: c * 2 * S + (p + 1) * S],
                                    start=True,
                                    stop=True,
                                    tile_position=(r0, 0),
                                )
                        E_sb = Ebufp.tile([128, 4 * S], BF16, tag="E")
                        nc.scalar.activation(E_sb[:], psS[:], AF.Exp, scale=SCALE)
                        return E_sb

                    def emit_av_norm(p, hp, E_sb):
                        # psO layout m-major: [m0ho0 | m0ho1 | m1ho0 | m1ho1]
                        psO = psSp.tile([128, 260], F32, tag="psS")
                        for m in range(2):
                            for ho in range(2):
                                h = 2 * hp + ho
                                for kk in range(2):
                                    nc.tensor.matmul(
                                        psO[:, m * 130 + ho * 65:
                                            m * 130 + ho * 65 + 65],
                                        E_sb[:, ho * 2 * S + kk * S + m * 128:
                                             ho * 2 * S + kk * S + (m + 1) * 128],
                                        vp_sb[:, kk * 520 + h * 65:
                                              kk * 520 + h * 65 + 65],
                                        start=(kk == 0),
                                        stop=(kk == 1),
                                    )
                        rcp = rbufp.tile([128, 4], F32, tag="rcp")
                        nc.vector.reciprocal(
                            rcp[:].rearrange("p (j o) -> p j o", o=1),
                            psO[:].rearrange("p (j c) -> p j c", c=65)[:, :, 64:65],
                        )
                        col = p * EMB + hp * 128
                        for m in range(2):  # batched normalize on DVE
                            rv = rcp[:, m * 2:m * 2 + 2]
                            rbc = bass.AP(
                                tensor=rv.tensor, offset=rv.offset,
                                ap=[rv.ap[0], rv.ap[1], [0, 64]],
                            )
                            nc.vector.tensor_mul(
                                cats[m][:, col:col + 128].rearrange(
                                    "p (ho c) -> p ho c", c=64),
                                psO[:, m * 130:m * 130 + 130].rearrange(
                                    "p (ho c) -> p ho c", c=65)[:, :, 0:64],
                                rbc,
                            )

                    pend = None
                    for p in range(2):
                        for hp in range(4):
                            E_sb = emit_scores(p, hp)
                            if pend is not None:
                                emit_av_norm(*pend)
                            pend = (p, hp, E_sb)
                    emit_av_norm(*pend)

                    # ---- cat -> cat^T ----
                    if DMA_TRANSPOSE:
                        for j in range(8):
                            for m in range(2):
                                nc.sync.dma_start(
                                    out=ct_sb[:, j * 2 * S + bo * S + m * 128:
                                              j * 2 * S + bo * S + (m + 1) * 128],
                                    in_=cats[m][:, j * 128:(j + 1) * 128],
                                    transpose=True,
                                )
                    else:
                        for j in range(8):
                            psT = psSp.tile([128, 256], BF16, tag="psS")
                            for m in range(2):
                                nc.tensor.transpose(
                                    psT[:, m * 128:(m + 1) * 128],
                                    cats[m][:, j * 128:(j + 1) * 128],
                                    id_sb[:],
                                )
                            eng = nc.scalar if j % 2 == 0 else nc.vector
                            if j % 2 == 0:
                                nc.scalar.activation(
                                    ct_sb[:, j * 2 * S + bo * S:
                                          j * 2 * S + bo * S + 256],
                                    psT[:], AF.Copy,
                                )
                            else:
                                nc.vector.tensor_copy(
                                    ct_sb[:, j * 2 * S + bo * S:
                                          j * 2 * S + bo * S + 256],
                                    psT[:],
                                )

                # ---- projection for the block pair: out^T [6, 2S] ----
                psP = psSp.tile([6, 2 * S], F32, tag="psS")
                for j in range(8):
                    nc.tensor.matmul(
                        psP[:],
                        wpt_sb[:, j * 6:(j + 1) * 6],
                        ct_sb[:, j * 2 * S:(j + 1) * 2 * S],
                        start=(j == 0),
                        stop=(j == 7),
                    )
                o_sb = obufp.tile([6, 2 * S], F32, tag="o")
                nc.vector.tensor_scalar_add(o_sb[:], psP[:], bpc_sb[:])
                nc.scalar.dma_start(
                    out=out_d[2 * bp_:2 * bp_ + 2].rearrange("b c t -> c b t"),
                    in_=o_sb[:].rearrange("c (b t) -> c b t", b=2),
                )

    nc.compile()
    return nc


_NC = None
TRACE = False  # set True (e.g. from test.py) to capture an NTFF profile


def _get_nc():
    global _NC
    if _NC is None:
        _NC = _build()
    return _NC


def _split16(x):
    B, C, H, W = x.shape
    nh, nw = H // BLK, W // BLK
    x = x.reshape(B, C, nh, BLK, nw, BLK).transpose(0, 2, 4, 1, 3, 5)
    return x.reshape(B * nh * nw, C, BLK, BLK)


def _combine16(x, H, W):
    nh, nw = H // BLK, W // BLK
    B = x.shape[0] // (nh * nw)
    C = x.shape[1]
    x = x.reshape(B, nh, nw, C, BLK, BLK).transpose(0, 3, 1, 4, 2, 5)
    return x.reshape(B, C, H, W)


def kernel(
    img1, img2, W_emb, b_emb, W_emb2, b_emb2, Wq, bq, Wk, bk, Wv, bv, Wp, bp
):
    img1 = np.asarray(img1, dtype=np.float32)
    img2 = np.asarray(img2, dtype=np.float32)
    bf = ml_dtypes.bfloat16

    # ---- host-side layout (pure reshapes/concats; no compute) ----
    x1t = _split16(img1).reshape(-1, 6, S)  # [512, 6, 256] channel-major
    x2t = _split16(img2).reshape(-1, 6, S)
    Bp = x1t.shape[0]
    ones = np.ones((Bp, 1, S), np.float32)
    x1a = np.concatenate([x1t, ones], axis=1)  # [512, 7, 256]
    x2a = np.concatenate([x2t, ones], axis=1)
    x12 = np.stack([x1a, x2a], axis=2).astype(bf)  # [512, 7, 2, 256]
    xc = np.concatenate([x1t, x2t, ones], axis=1).astype(bf)  # [512, 13, 256]

    wemb1 = np.concatenate(
        [np.asarray(W_emb, np.float32), np.asarray(b_emb, np.float32)[None, :]], 0
    ).astype(bf)  # [7, 512]
    wemb2 = np.concatenate(
        [np.asarray(W_emb2, np.float32), np.asarray(b_emb2, np.float32)[None, :]], 0
    ).astype(bf)  # [13, 512]

    def wlay(w):  # [512, 512] -> [128, 4*512] with [p, k*512+o] = w[k*128+p, o]
        return (
            np.asarray(w, np.float32)
            .reshape(4, 128, EMB)
            .transpose(1, 0, 2)
            .reshape(128, 4 * EMB)
            .astype(bf)
        )

    wq_h, wk_h, wv_h = wlay(Wq), wlay(Wk), wlay(Wv)
    wpt_h = (
        np.asarray(Wp, np.float32)
        .T.reshape(8, 128, 6)
        .transpose(1, 0, 2)
        .reshape(128, 48)
        .astype(bf)
    )
    bqk_h = np.concatenate(
        [
            np.asarray(bq, np.float32).reshape(4, 128).T,
            np.asarray(bk, np.float32).reshape(4, 128).T,
        ],
        axis=1,
    )  # [128, 8]
    bvb_h = np.ascontiguousarray(
        np.broadcast_to(np.asarray(bv, np.float32), (128, EMB))
    )
    bpc_h = np.asarray(bp, np.float32).reshape(6, 1)
    id_h = np.eye(128, dtype=np.float32).astype(bf)

    nc = _get_nc()
    core_ids = list(range(NCORES))
    in_maps = []
    for c in range(NCORES):
        sl = slice(c * NBLK, (c + 1) * NBLK)
        in_maps.append({
            "x12": np.ascontiguousarray(x12[sl]).reshape(NBLK, 7, 2 * S),
            "xc": np.ascontiguousarray(xc[sl]),
            "wq": wq_h, "wk": wk_h, "wv": wv_h,
            "we1": wemb1, "we2": wemb2, "wpt": wpt_h,
            "bqk": bqk_h, "bvb": bvb_h, "bpc": bpc_h, "ident": id_h,
        })
    res = run_bass_kernel_spmd(nc, in_maps, core_ids, trace=TRACE)
    if TRACE and res.exec_time_ns is not None:
        print(f"HW exec time: {res.exec_time_ns} ns")
    out = np.concatenate([res.results[c]["out"] for c in range(NCORES)], axis=0)
    return _combine16(out.reshape(Bp, 6, BLK, BLK), 128, 128)


# revision 13
# speedup vs baseline: 1.3946x; 1.3946x over previous
"""Cross-MultiAttention Trainium2 kernel.

Reference computation (nn_Cross_MultiAttention): two [8,6,128,128] images are
split into 16x16 blocks (B'=512 independent blocks of S=256 tokens, C=6
channels), embedded to EMB=512, cross-attended (two query sets vs a shared
K/V from the concatenated features, 8 heads, depth 64, scale EMB^-0.5), the
two attention outputs are concatenated channel-wise and projected back to 6
channels with a 1x1 conv, then blocks are reassembled.

Strategy: data-parallel over blocks — 64 blocks per NeuronCore x 8 cores.
All device compute is bf16-in/fp32-accumulate on the TensorEngine. Layouts
are feature-major ("transposed") end-to-end so no data transposes are needed
until the final projection:
  e^T = Wemb'^T @ x'      (x' carries a ones-row => embedding bias free;
                           the two images share Wemb so they ride one
                           N=512 moving operand)
  Q^T/K^T feature-major; V token-major with a ones-column per head so the
  attention-value matmul also emits the softmax denominator.
  scores^T = K_h^T' @ Q_h — head pairs issued into disjoint PE row groups
  (tile_position (0,0)/(64,0)) so two K=64 matmuls share the array.
  exp on ScalarE (scale folded in), one op per head-pair.
  O = E^T V' (q-major) -> per-partition reciprocal+scale on VectorE.
  concat -> DMA-engine transposes -> out^T = Wp^T-chunks @ cat^T.
Biases bq/bk (per-partition in feature-major) and bp ride the PSUM->SBUF
copies as tensor_scalar adds; bv (per-free in token-major) is a
tensor_tensor add against a broadcast tile. All are exact.
"""

import numpy as np
import ml_dtypes

import concourse.bass as bass
import concourse.mybir as mybir
import concourse.tile as tile
from concourse import bacc
from concourse.bass_utils import run_bass_kernel_spmd

BLK = 16
EMB = 512
HEADS = 8
DEPTH = 64
S = 256  # tokens per block (16*16)
SCALE = EMB ** (-0.5)
NBLK = 64  # blocks per core
NCORES = 8

BF16 = mybir.dt.bfloat16
F32 = mybir.dt.float32
AF = mybir.ActivationFunctionType

DMA_TRANSPOSE = False  # cat->cat^T on DMA engines instead of the PE array


def _build():
    nc = bacc.Bacc(None)

    # ---- DRAM parameters (per core) ----
    x12_d = nc.declare_dram_parameter("x12", [NBLK, 7, 2 * S], BF16, isOutput=False)
    xc_d = nc.declare_dram_parameter("xc", [NBLK, 13, S], BF16, isOutput=False)
    wq_d = nc.declare_dram_parameter("wq", [128, 4 * EMB], BF16, isOutput=False)
    wk_d = nc.declare_dram_parameter("wk", [128, 4 * EMB], BF16, isOutput=False)
    wv_d = nc.declare_dram_parameter("wv", [128, 4 * EMB], BF16, isOutput=False)
    we1_d = nc.declare_dram_parameter("we1", [7, EMB], BF16, isOutput=False)
    we2_d = nc.declare_dram_parameter("we2", [13, EMB], BF16, isOutput=False)
    wpt_d = nc.declare_dram_parameter("wpt", [128, 48], BF16, isOutput=False)
    bqk_d = nc.declare_dram_parameter("bqk", [128, 8], F32, isOutput=False)
    bvb_d = nc.declare_dram_parameter("bvb", [128, EMB], F32, isOutput=False)
    bpc_d = nc.declare_dram_parameter("bpc", [6, 1], F32, isOutput=False)
    id_d = nc.declare_dram_parameter("ident", [128, 128], BF16, isOutput=False)
    out_d = nc.declare_dram_parameter("out", [NBLK, 6, S], F32, isOutput=True)

    with tile.TileContext(nc) as tc:
        with (
            tc.tile_pool(name="const", bufs=1) as constp,
            tc.tile_pool(name="xin", bufs=3) as xinp,
            tc.tile_pool(name="ebuf", bufs=3) as ebufp,
            tc.tile_pool(name="qkbuf", bufs=3) as qkbufp,
            tc.tile_pool(name="vbuf", bufs=2) as vbufp,
            tc.tile_pool(name="Ebuf", bufs=3) as Ebufp,
            tc.tile_pool(name="catbuf", bufs=3) as catbufp,
            tc.tile_pool(name="ctbuf", bufs=2) as ctbufp,
            tc.tile_pool(name="rbuf", bufs=4) as rbufp,
            tc.tile_pool(name="obuf", bufs=3) as obufp,
            tc.tile_pool(name="psA", bufs=2, space="PSUM") as psAp,
            tc.tile_pool(name="psS", bufs=2, space="PSUM") as psSp,
        ):
            # ---- constants into SBUF ----
            wq_sb = constp.tile([128, 4 * EMB], BF16, tag="wq")
            wk_sb = constp.tile([128, 4 * EMB], BF16, tag="wk")
            wv_sb = constp.tile([128, 4 * EMB], BF16, tag="wv")
            we1_sb = constp.tile([7, EMB], BF16, tag="we1")
            we2_sb = constp.tile([13, EMB], BF16, tag="we2")
            wpt_sb = constp.tile([128, 48], BF16, tag="wpt")
            bqk_sb = constp.tile([128, 8], F32, tag="bqk")
            bvb_sb = constp.tile([128, EMB], F32, tag="bvb")
            bpc_sb = constp.tile([6, 1], F32, tag="bpc")
            id_sb = constp.tile([128, 128], BF16, tag="ident")

            nc.sync.dma_start(out=wq_sb[:], in_=wq_d[:])
            nc.sync.dma_start(out=wk_sb[:], in_=wk_d[:])
            nc.sync.dma_start(out=wv_sb[:], in_=wv_d[:])
            nc.sync.dma_start(out=we1_sb[:], in_=we1_d[:])
            nc.sync.dma_start(out=we2_sb[:], in_=we2_d[:])
            nc.sync.dma_start(out=wpt_sb[:], in_=wpt_d[:])
            nc.sync.dma_start(out=bqk_sb[:], in_=bqk_d[:])
            nc.sync.dma_start(out=bvb_sb[:], in_=bvb_d[:])
            nc.sync.dma_start(out=bpc_sb[:], in_=bpc_d[:])
            nc.sync.dma_start(out=id_sb[:], in_=id_d[:])

            for bp_ in range(NBLK // 2):  # block pairs (projection batched)
                ct_sb = ctbufp.tile([128, 8 * 2 * S], BF16, tag="ct")
                for bo in range(2):
                    b = 2 * bp_ + bo
                    x12_sb = xinp.tile([7, 2 * S], BF16, tag="x12")
                    xc_sb = xinp.tile([13, S], BF16, tag="xc")
                    nc.sync.dma_start(out=x12_sb[:], in_=x12_d[b])
                    nc.sync.dma_start(out=xc_sb[:], in_=xc_d[b])

                    # ---- embeddings (feature-major) ----
                    # e12 chunk k = [e1_k | e2_k] (the two images share Wemb)
                    e12_sb = ebufp.tile([128, 4 * 2 * S], BF16, tag="e12")
                    for half in range(2):
                        ps = psAp.tile([128, 2 * 2 * S], F32, tag="psA")
                        for mm in range(2):
                            m = 2 * half + mm
                            nc.tensor.matmul(
                                ps[:, mm * 2 * S:(mm + 1) * 2 * S],
                                we1_sb[:, m * 128:(m + 1) * 128],
                                x12_sb[:],
                                start=True,
                                stop=True,
                            )
                        if half == 0:
                            nc.scalar.activation(
                                e12_sb[:, half * 4 * S:(half + 1) * 4 * S],
                                ps[:], AF.Copy,
                            )
                        else:
                            nc.vector.tensor_copy(
                                e12_sb[:, half * 4 * S:(half + 1) * 4 * S], ps[:]
                            )
                    ec_sb = ebufp.tile([128, 4 * S], BF16, tag="ec")
                    psc = psAp.tile([128, 2 * 2 * S], F32, tag="psA")
                    for m in range(4):
                        nc.tensor.matmul(
                            psc[:, m * S:(m + 1) * S],
                            we2_sb[:, m * 128:(m + 1) * 128],
                            xc_sb[:],
                            start=True,
                            stop=True,
                        )
                    nc.scalar.activation(ec_sb[:], psc[:], AF.Copy)

                    # ---- Q1|Q2 (feature-major), K (feature-major) ----
                    q12_sb = qkbufp.tile([128, 4 * 2 * S], BF16, tag="q12")
                    for half in range(2):
                        ps = psAp.tile([128, 2 * 2 * S], F32, tag="psA")
                        for mm in range(2):
                            m = 2 * half + mm
                            for k in range(4):
                                nc.tensor.matmul(
                                    ps[:, mm * 2 * S:(mm + 1) * 2 * S],
                                    wq_sb[:, k * EMB + m * 128:
                                          k * EMB + (m + 1) * 128],
                                    e12_sb[:, k * 2 * S:(k + 1) * 2 * S],
                                    start=(k == 0),
                                    stop=(k == 3),
                                )
                        for mm in range(2):
                            m = 2 * half + mm
                            nc.vector.tensor_scalar_add(
                                q12_sb[:, m * 2 * S:(m + 1) * 2 * S],
                                ps[:, mm * 2 * S:(mm + 1) * 2 * S],
                                bqk_sb[:, m:m + 1],
                            )

                    k_sb = qkbufp.tile([128, 4 * S], BF16, tag="k")
                    psk = psAp.tile([128, 2 * 2 * S], F32, tag="psA")
                    for m in range(4):
                        for k in range(4):
                            nc.tensor.matmul(
                                psk[:, m * S:(m + 1) * S],
                                wk_sb[:, k * EMB + m * 128: k * EMB + (m + 1) * 128],
                                ec_sb[:, k * S:(k + 1) * S],
                                start=(k == 0),
                                stop=(k == 3),
                            )
                    for m in range(4):
                        nc.vector.tensor_scalar_add(
                            k_sb[:, m * S:(m + 1) * S],
                            psk[:, m * S:(m + 1) * S],
                            bqk_sb[:, 4 + m:5 + m],
                        )

                    # ---- V token-major, ones column per head ----
                    psV = psAp.tile([128, 2 * 2 * S], F32, tag="psA")
                    for t in range(2):
                        for k in range(4):
                            nc.tensor.matmul(
                                psV[:, t * EMB:(t + 1) * EMB],
                                ec_sb[:, k * S + t * 128: k * S + t * 128 + 128],
                                wv_sb[:, k * EMB:(k + 1) * EMB],
                                start=(k == 0),
                                stop=(k == 3),
                            )
                    vp_sb = vbufp.tile([128, 2 * 520], BF16, tag="vp")
                    nc.vector.memset(
                        vp_sb[:].rearrange(
                            "p (t h c) -> p t h c", t=2, h=8
                        )[:, :, :, 64],
                        1.0,
                    )
                    for t in range(2):
                        nc.vector.tensor_add(
                            vp_sb[:, t * 520:(t + 1) * 520].rearrange(
                                "p (h c) -> p h c", c=65
                            )[:, :, 0:64],
                            psV[:, t * EMB:(t + 1) * EMB].rearrange(
                                "p (h c) -> p h c", c=64
                            ),
                            bvb_sb[:].rearrange("p (h c) -> p h c", c=64),
                        )

                    # ---- attention: head pairs in disjoint PE row groups,
                    # software-pipelined: scores/exp of pair N+1 issue
                    # before the attention-value matmuls of pair N ----
                    cat0 = catbufp.tile([128, 2 * EMB], BF16, tag="cat0")
                    cat1 = catbufp.tile([128, 2 * EMB], BF16, tag="cat1")
                    cats = (cat0, cat1)

                    def emit_scores(p, hp):
                        c = hp  # feature chunk index = h//2
                        psS = psSp.tile([128, 4 * S], F32, tag="psS")
                        for kk in range(2):
                            for ho in range(2):
                                r0 = ho * 64
                                nc.tensor.matmul(
                                    psS[:, ho * 2 * S + kk * S:
                                        ho * 2 * S + (kk + 1) * S],
                                    k_sb[r0:r0 + 64,
                                         c * S + kk * 128: c * S + (kk + 1) * 128],
                                    q12_sb[r0:r0 + 64,
                                           c * 2 * S + p * S: c * 2 * S + (p + 1) * S],
                                    start=True,
                                    stop=True,
                                    tile_position=(r0, 0),
                                )
                        E_sb = Ebufp.tile([128, 4 * S], BF16, tag="E")
                        nc.scalar.activation(E_sb[:], psS[:], AF.Exp, scale=SCALE)
                        return E_sb

                    def emit_av_norm(p, hp, E_sb):
                        # psO layout m-major: [m0ho0 | m0ho1 | m1ho0 | m1ho1]
                        psO = psSp.tile([128, 260], F32, tag="psS")
                        for m in range(2):
                            for ho in range(2):
                                h = 2 * hp + ho
                                for kk in range(2):
                                    nc.tensor.matmul(
                                        psO[:, m * 130 + ho * 65:
                                            m * 130 + ho * 65 + 65],
                                        E_sb[:, ho * 2 * S + kk * S + m * 128:
                                             ho * 2 * S + kk * S + (m + 1) * 128],
                                        vp_sb[:, kk * 520 + h * 65:
                                              kk * 520 + h * 65 + 65],
                                        start=(kk == 0),
                                        stop=(kk == 1),
                                    )
                        rcp = rbufp.tile([128, 4], F32, tag="rcp")
                        nc.vector.reciprocal(
                            rcp[:].rearrange("p (j o) -> p j o", o=1),
                            psO[:].rearrange("p (j c) -> p j c", c=65)[:, :, 64:65],
                        )
                        col = p * EMB + hp * 128
                        for m in range(2):  # batched normalize on DVE
                            rv = rcp[:, m * 2:m * 2 + 2]
                            rbc = bass.AP(
                                tensor=rv.tensor, offset=rv.offset,
                                ap=[rv.ap[0], rv.ap[1], [0, 64]],
                            )
                            nc.vector.tensor_mul(
                                cats[m][:, col:col + 128].rearrange(
                                    "p (ho c) -> p ho c", c=64),
                                psO[:, m * 130:m * 130 + 130].rearrange(
                                    "p (ho c) -> p ho c", c=65)[:, :, 0:64],
                                rbc,
                            )

                    pend = None
                    for p in range(2):
                        for hp in range(4):
                            E_sb = emit_scores(p, hp)
                            if pend is not None:
                                emit_av_norm(*pend)
                            pend = (p, hp, E_sb)
                    emit_av_norm(*pend)

                    # ---- cat -> cat^T ----
                    if DMA_TRANSPOSE:
                        for j in range(8):
                            for m in range(2):
                                nc.sync.dma_start(
                                    out=ct_sb[:, j * 2 * S + bo * S + m * 128:
                                              j * 2 * S + bo * S + (m + 1) * 128],
                                    in_=cats[m][:, j * 128:(j + 1) * 128],
                                    transpose=True,
                                )
                    else:
                        psT = psSp.tile([128, 2048], BF16, tag="psS")
                        for j in range(8):
                            for m in range(2):
                                nc.tensor.transpose(
                                    psT[:, j * 256 + m * 128:
                                        j * 256 + (m + 1) * 128],
                                    cats[m][:, j * 128:(j + 1) * 128],
                                    id_sb[:],
                                )
                        nc.vector.tensor_copy(
                            ct_sb[:].rearrange("p (j t) -> p j t", t=2 * S)[
                                :, :, bo * S:(bo + 1) * S],
                            psT[:].rearrange("p (j t) -> p j t", t=S),
                        )

                # ---- projection for the block pair: out^T [6, 2S] ----
                psP = psSp.tile([6, 2 * S], F32, tag="psS")
                for j in range(8):
                    nc.tensor.matmul(
                        psP[:],
                        wpt_sb[:, j * 6:(j + 1) * 6],
                        ct_sb[:, j * 2 * S:(j + 1) * 2 * S],
                        start=(j == 0),
                        stop=(j == 7),
                    )
                o_sb = obufp.tile([6, 2 * S], F32, tag="o")
                nc.vector.tensor_scalar_add(o_sb[:], psP[:], bpc_sb[:])
                nc.sync.dma_start(
                    out=out_d[2 * bp_:2 * bp_ + 2].rearrange("b c t -> c b t"),
                    in_=o_sb[:].rearrange("c (b t) -> c b t", b=2),
                )

    nc.compile()
    return nc


_NC = None
TRACE = False  # set True (e.g. from test.py) to capture an NTFF profile


def _get_nc():
    global _NC
    if _NC is None:
        _NC = _build()
    return _NC


def _split16(x):
    B, C, H, W = x.shape
    nh, nw = H // BLK, W // BLK
    x = x.reshape(B, C, nh, BLK, nw, BLK).transpose(0, 2, 4, 1, 3, 5)
    return x.reshape(B * nh * nw, C, BLK, BLK)


def _combine16(x, H, W):
    nh, nw = H // BLK, W // BLK
    B = x.shape[0] // (nh * nw)
    C = x.shape[1]
    x = x.reshape(B, nh, nw, C, BLK, BLK).transpose(0, 3, 1, 4, 2, 5)
    return x.reshape(B, C, H, W)


def kernel(
    img1, img2, W_emb, b_emb, W_emb2, b_emb2, Wq, bq, Wk, bk, Wv, bv, Wp, bp
):
    img1 = np.asarray(img1, dtype=np.float32)
    img2 = np.asarray(img2, dtype=np.float32)
    bf = ml_dtypes.bfloat16

    # ---- host-side layout (pure reshapes/concats; no compute) ----
    x1t = _split16(img1).reshape(-1, 6, S)  # [512, 6, 256] channel-major
    x2t = _split16(img2).reshape(-1, 6, S)
    Bp = x1t.shape[0]
    ones = np.ones((Bp, 1, S), np.float32)
    x1a = np.concatenate([x1t, ones], axis=1)  # [512, 7, 256]
    x2a = np.concatenate([x2t, ones], axis=1)
    x12 = np.stack([x1a, x2a], axis=2).astype(bf)  # [512, 7, 2, 256]
    xc = np.concatenate([x1t, x2t, ones], axis=1).astype(bf)  # [512, 13, 256]

    wemb1 = np.concatenate(
        [np.asarray(W_emb, np.float32), np.asarray(b_emb, np.float32)[None, :]], 0
    ).astype(bf)  # [7, 512]
    wemb2 = np.concatenate(
        [np.asarray(W_emb2, np.float32), np.asarray(b_emb2, np.float32)[None, :]], 0
    ).astype(bf)  # [13, 512]

    def wlay(w):  # [512, 512] -> [128, 4*512] with [p, k*512+o] = w[k*128+p, o]
        return (
            np.asarray(w, np.float32)
            .reshape(4, 128, EMB)
            .transpose(1, 0, 2)
            .reshape(128, 4 * EMB)
            .astype(bf)
        )

    wq_h, wk_h, wv_h = wlay(Wq), wlay(Wk), wlay(Wv)
    wpt_h = (
        np.asarray(Wp, np.float32)
        .T.reshape(8, 128, 6)
        .transpose(1, 0, 2)
        .reshape(128, 48)
        .astype(bf)
    )
    bqk_h = np.concatenate(
        [
            np.asarray(bq, np.float32).reshape(4, 128).T,
            np.asarray(bk, np.float32).reshape(4, 128).T,
        ],
        axis=1,
    )  # [128, 8]
    bvb_h = np.ascontiguousarray(
        np.broadcast_to(np.asarray(bv, np.float32), (128, EMB))
    )
    bpc_h = np.asarray(bp, np.float32).reshape(6, 1)
    id_h = np.eye(128, dtype=np.float32).astype(bf)

    nc = _get_nc()
    core_ids = list(range(NCORES))
    in_maps = []
    for c in range(NCORES):
        sl = slice(c * NBLK, (c + 1) * NBLK)
        in_maps.append({
            "x12": np.ascontiguousarray(x12[sl]).reshape(NBLK, 7, 2 * S),
            "xc": np.ascontiguousarray(xc[sl]),
            "wq": wq_h, "wk": wk_h, "wv": wv_h,
            "we1": wemb1, "we2": wemb2, "wpt": wpt_h,
            "bqk": bqk_h, "bvb": bvb_h, "bpc": bpc_h, "ident": id_h,
        })
    res = run_bass_kernel_spmd(nc, in_maps, core_ids, trace=TRACE)
    if TRACE and res.exec_time_ns is not None:
        print(f"HW exec time: {res.exec_time_ns} ns")
    out = np.concatenate([res.results[c]["out"] for c in range(NCORES)], axis=0)
    return _combine16(out.reshape(Bp, 6, BLK, BLK), 128, 128)
